# revision 37
# baseline (speedup 1.0000x reference)
"""Trainium2 Bass kernel for the DGGGL graph-conv GRU cell (gnn_message_passing).

Strategy: data-parallel over batch B=8 across the 8 NeuronCores (one batch
element per core, no collectives).  The heavy work is 8 products A @ X with
A [2048, 2048] per (support, cheb-step, agcn): we host-pretranspose A so the
device computes (A @ X)^T = X^T A^T with the small [128, 66] feature tile as
the PE-stationary operand and A^T streamed at F=512 (fp8 DoubleRow).  Both
supports' A^T live resident in SBUF as e4m3 (32KB/partition each), loaded
from HBM in ONE coalesced DMA per support (per-DMA HWDGE issue overhead
otherwise serializes into ~30us of dead time).

Chebyshev fold: T2 = 2*A@T1 - X never feeds another product (K=3), so the
kernel only computes P2 := A@T1 and folds the -X into the src-slot dense
weights (W0' = W0 - W2) and the 2x into the P2-slot weights.  T1/P2 land as
fp8 pairs u1=32*T1, u2=64*P2 interleaved in one [C, 2, N] tile, which is
exactly the moving AP fp8 DoubleRow wants: the 5-term dense matmul becomes
1 bf16 MM (src) + 2 fp8-DR MMs.  Dense weights ship as fp8 scaled by SW=64;
the eviction activation divides it back out before bias.

Everything accumulates in fp32 PSUM; elementwise chains are bf16
(validated ~8e-3 max-rel vs the fp32 reference on the host).

Feature order on device is [s1 (0:32), s2 (32:64), xt (64:66)] so that
partition-dim slices land on 32-aligned boundaries (HW requirement); the
host permutes weight rows to match.

"""

import contextlib

import numpy as np
import ml_dtypes

import concourse.bass as bass
import concourse.mybir as mybir
import concourse.tile as tile
from concourse import bacc
from concourse.bass_utils import run_bass_kernel_spmd
from concourse.masks import make_identity

BF16 = mybir.dt.bfloat16
F8 = mybir.dt.float8e4
F32 = mybir.dt.float32
AF = mybir.ActivationFunctionType
DR = mybir.MatmulPerfMode.DoubleRow
SWI = mybir.MatmulPerfMode.DoubleRowSwInterleave

P = 128          # partitions
N = 2048         # nodes
NK = N // P      # 16 k-chunks
C = 66           # feature dim into the AGCNs (2 + 32 + 32)
CP = 256         # SWI stationary slot bytes per kp-pair (2 x 128-col planes)
CIN = 74         # MLP gate input dim
G = 96           # 3 * DO
DO = 32
FB = 512         # dense matmul moving free-dim chunk (one PSUM bank of fp32)
NF = N // FB     # 4
PFB = 512        # product moving free-dim chunk (DR caps moving AP at 2x512)
NPF = N // PFB   # 4
KPF = PFB // P   # k-chunks per product F-chunk (4)
NCORES = 8
UNROLL = 4       # loop-body unroll (banks alternate mod 2)

# fp8 scaling (exact powers of two): A is shipped as A*SA in e4m3 (softmax
# values <=1 so max 64 < 240); terms are stored as u1 = 32*T1, u2 = 64*P2;
# dense-slot fp8 weights carry SW (evict activation scales by 1/SW).
SA = 64.0
ST = 32.0        # u1 = ST * T1
SW = 64.0

_NC_CACHE = None


def _build_bass(reps=1, hw_loop=True, staggered=False):
    # staggered=True (For_i staggered_reset + stage boundaries) measured
    # 94.3us/iter on HW vs 89.3us for the plain barrier loop — the rolling
    # stage postambles cost more than the per-trip barrier they replace.
    # hw_loop=False python-unrolls the reps (no For_i) — TimelineSim can't
    # resolve For_i's register branches, so sims use the unrolled form.
    nc = bacc.Bacc("TRN2", target_bir_lowering=False, debug=False)

    # ---- DRAM I/O -------------------------------------------------------
    # aT/xnat are shipped in SBUF-tile layout ([P, NK, ...] chunked, flat per
    # partition) so each load is ONE maximally-coalesced DMA.
    aT_d = nc.dram_tensor("aT", [2, P, NK * N], F8, kind="ExternalInput")
    xnat_d = nc.dram_tensor("xnat", [P, (NK // 2) * CP], F8, kind="ExternalInput")
    gin_d = nc.dram_tensor("gin", [CIN, N], BF16, kind="ExternalInput")
    sdiff_d = nc.dram_tensor("sdiff", [DO, N], BF16, kind="ExternalInput")
    mlpw_d = nc.dram_tensor("mlp_w", [CIN, DO], BF16, kind="ExternalInput")
    gsrc_d = nc.dram_tensor("gsrc", [C, G], BF16, kind="ExternalInput")
    gpair_d = nc.dram_tensor("gpair", [C, 2 * 2 * G], F8, kind="ExternalInput")
    usrc_d = nc.dram_tensor("usrc", [C, DO], BF16, kind="ExternalInput")
    upair_d = nc.dram_tensor("upair", [C, 2 * 2 * DO], F8, kind="ExternalInput")
    hopw_d = nc.dram_tensor("hop_w", [DO, DO], BF16, kind="ExternalInput")
    mlpb_d = nc.dram_tensor("mlp_b", [DO, 1], F32, kind="ExternalInput")
    gateb_d = nc.dram_tensor("gate_b", [G, 1], F32, kind="ExternalInput")
    updb_d = nc.dram_tensor("upd_b", [DO, 1], F32, kind="ExternalInput")
    hopb_d = nc.dram_tensor("hop_b", [DO, 1], F32, kind="ExternalInput")
    hT_d = nc.dram_tensor("h_T", [DO, N], BF16, kind="ExternalOutput")
    tnT_d = nc.dram_tensor("tn_T", [DO, N], BF16, kind="ExternalOutput")

    with tile.TileContext(nc) as tc:
        with (
            tc.tile_pool(name="const", bufs=1) as const,
            tc.tile_pool(name="abuf", bufs=1) as abuf,
            tc.tile_pool(name="natp", bufs=1) as natp,
            tc.tile_pool(name="tnatp", bufs=2) as tnatp,
            tc.tile_pool(name="termp", bufs=1) as termp,
            tc.tile_pool(name="f32p", bufs=1) as f32p,
            tc.tile_pool(name="pp", bufs=4, space="PSUM") as pp,
            tc.tile_pool(name="tp", bufs=2, space="PSUM") as tp,
            tc.tile_pool(name="dp", bufs=2, space="PSUM") as dp,
        ):
            # ---- constants / weights (loaded once, outside the loop) ---
            ident = const.tile([P, P], BF16)
            make_identity(nc, ident)
            ident8 = const.tile([P, P], F8)
            make_identity(nc, ident8)
            mlpw = const.tile([CIN, DO], BF16)
            nc.sync.dma_start(out=mlpw, in_=mlpw_d[:, :])
            gsrc = const.tile([C, G], BF16)
            nc.sync.dma_start(out=gsrc, in_=gsrc_d[:, :])
            gpair = const.tile([C, 2, 2 * G], F8)
            nc.sync.dma_start(out=gpair, in_=gpair_d[:, :])
            usrc = const.tile([C, DO], BF16)
            nc.sync.dma_start(out=usrc, in_=usrc_d[:, :])
            upair = const.tile([C, 2, 2 * DO], F8)
            nc.sync.dma_start(out=upair, in_=upair_d[:, :])
            hopw = const.tile([DO, DO], BF16)
            nc.sync.dma_start(out=hopw, in_=hopw_d[:, :])
            mlpb = const.tile([DO, 1], F32)
            gateb = const.tile([G, 1], F32)
            updb = const.tile([DO, 1], F32)
            hopb = const.tile([DO, 1], F32)
            nc.sync.dma_start(out=mlpb, in_=mlpb_d[:, :])
            nc.sync.dma_start(out=gateb, in_=gateb_d[:, :])
            nc.sync.dma_start(out=updb, in_=updb_d[:, :])
            nc.sync.dma_start(out=hopb, in_=hopb_d[:, :])

            # Unrolled loop body with input-tile banks alternating mod 2:
            # iteration i+1's A/gin/xnat DMA stream overlaps iteration i's
            # compute phase (software pipelining across loop iterations).
            assert reps == 1 or reps % UNROLL == 0, f"reps must be 1 or %{UNROLL}"
            args = (
                nc, tc, aT_d, xnat_d, gin_d, sdiff_d, hT_d, tnT_d,
                ident, ident8, mlpw, gsrc, gpair, usrc, upair, hopw,
                mlpb, gateb, updb, hopb,
                const, abuf, natp, tnatp, termp, f32p, pp, tp, dp,
            )
            if reps == 1:
                _emit_body(*args, bank=0)
            elif not hw_loop:
                for u in range(reps):
                    _emit_body(*args, bank=u % 2)
            else:
                with tc.For_i(0, reps // UNROLL, staggered_reset=staggered):
                    for u in range(UNROLL):
                        if staggered and u:
                            tc.stage_boundary()
                        _emit_body(*args, bank=u % 2)

    nc.compile()
    return nc


def _emit_body(nc, tc, aT_d, xnat_d, gin_d, sdiff_d, hT_d, tnT_d,
               ident, ident8, mlpw, gsrc, gpair, usrc, upair, hopw,
               mlpb, gateb, updb, hopb,
               const, abuf, natp, tnatp, termp, f32p, pp, tp, dp, bank=0):
    # ---- activations in (small; issued first so the T1 stationary and mlp
    # inputs land before the big A streams start).  Input tiles are tagged
    # per unroll bank so the next iteration's loads overlap this one's
    # compute. gin feature order is [s2, s1, xt, ge] (host-permuted) so s2
    # sits at partitions 0:32, partition-aligned with mr/state for the DVE.
    gin = const.tile([CIN, N], BF16, tag=f"gin{bank}", name="gin")
    nc.sync.dma_start(out=gin, in_=gin_d[:, :])
    xT = gin[0:C, :]          # X^T is the first 66 rows of gate_in^T
    sdiff = const.tile([DO, N], BF16, tag=f"sdiff{bank}", name="sdiff")
    nc.sync.dma_start(out=sdiff, in_=sdiff_d[:, :])
    xnat = natp.tile([P, NK // 2, CP], F8, tag=f"xnat{bank}", name="xnat")
    nc.sync.dma_start(out=xnat, in_=xnat_d[:, :])

    # ---- resident adjacency (transposed, fp8, pre-scaled by SA): 4 1MB
    # coalesced DMAs per support so T1's kp-accumulation pipelines with the
    # stream while per-DMA HWDGE issue overhead stays negligible.
    a_res = []
    for s in range(2):
        at = abuf.tile([P, NK, N], F8, tag=f"a{s}_{bank}", name=f"a{s}")
        for q in range(4):
            nc.sync.dma_start(
                out=at[:, 4 * q:4 * (q + 1), :],
                in_=aT_d[s, :, 4 * q * N:4 * (q + 1) * N],
            )
        a_res.append(at)

    # ---- MLP mixing gate + state = mr*(s1-s2) + s2 ----------------------
    mr = f32p.tile([DO, N], BF16, tag="mrhc", name="mr")
    for f in range(NPF):
        fs = slice(f * PFB, (f + 1) * PFB)
        ps = pp.tile([DO, PFB], F32, tag="pp", name="ps_mlp")
        nc.tensor.matmul(ps, mlpw, gin[:, fs], start=True, stop=True)
        nc.scalar.activation(mr[:, fs], ps, AF.Sigmoid, bias=mlpb)
    state = f32p.tile([DO, N], BF16, tag="state_tn", name="state")
    nc.vector.tensor_mul(state, mr, sdiff)
    nc.vector.tensor_add(state, state, gin[0:DO, :])

    # ---- helpers --------------------------------------------------------
    def product(nat_tile, s, evict, lag=None):
        """psum[c, f] = sum_k nat_chunk_k^T @ A_s^T[k, f]; evict(f, psum).
        fp8 DoubleRow: each matmul contracts a PAIR of 128-row chunks.
        lag(f) emits one F-chunk behind the matmuls so PE work (transposes)
        that waits on evict(f)'s ACT/DVE copy never stalls the PE queue."""
        for f in range(NPF):
            fs = slice(f * PFB, (f + 1) * PFB)
            ps = pp.tile([P, PFB], F32, tag="pp", name="ps_prod")
            for kp in range(NK // 2):
                nc.tensor.matmul(
                    ps,
                    nat_tile[:, kp, 0:CP],
                    a_res[s][:, 2 * kp:2 * kp + 2, fs],
                    start=(kp == 0),
                    stop=(kp == NK // 2 - 1),
                    perf_mode=SWI,
                )
            evict(f, ps[0:C, :])
            if lag is not None and f >= 1:
                lag(f - 1)
        if lag is not None:
            lag(NPF - 1)

    def transpose_to_nat(src_T, dst_nat, scale, f, fp8_src):
        """F-chunk f of src_T ([C, N]) -> SWI stationary slots of dst_nat
        ([P, NK//2, CP]): chunk-pair planes byte-interleaved at positions
        124+2p+i (tail-packed; the matching psum feature order is absorbed
        into host-side weight-row permutations).  The fp8 PE transpose
        writes element-step-2 natively; the bf16 path interleaves in the
        PSUM->SBUF copy instead.  Copies alternate ACT/DVE."""
        if fp8_src:
            # fp8 PE transpose writes element-step-2 at a 4-byte-aligned
            # base, so each chunk-plane lands in its own region; the copy
            # does the byte-interleave.
            tps5 = tp.tile([P, 2, 2, C, 2], F8, tag="tp", name="tps8")
            for j2 in range(2):
                for i in range(2):
                    k = f * KPF + 2 * j2 + i
                    nc.tensor.transpose(
                        tps5[:, j2, i, :, 0],
                        src_T[:, k * P:(k + 1) * P], ident8[0:C, 0:C],
                    )
            tin = tps5[:, :, :, :, 0].rearrange("p a i c -> p a c i")
        else:
            tps3 = tp.tile([P, 2, 2, C], BF16, tag="tp", name="tps")
            for j2 in range(2):
                for i in range(2):
                    k = f * KPF + 2 * j2 + i
                    nc.tensor.transpose(
                        tps3[:, j2, i, :],
                        src_T[:, k * P:(k + 1) * P], ident[0:C, 0:C],
                    )
            tin = tps3.rearrange("p a i c -> p a c i")
        tout = dst_nat[:, 2 * f:2 * f + 2, 124:CP]
        # NOTE: GpSimd cannot access PSUM (BIR verifier rejects it), so the
        # copies stay on DVE/ACT even though Pool is idle.
        if f % 2 == 0:
            nc.vector.tensor_scalar_mul(tout, tin, scale)
        else:
            nc.scalar.activation(tout, tin, AF.Copy, scale=scale)

    def agcn(src_nat, src_T, wsrc, wpair, out_cb):
        """Terms u1 = ST*A@src, u2 = 2*ST*A@(A@src) per support, fp8-pair
        interleaved in t12[s] [C, 2, N]; then the dense matmul = 1 bf16 MM
        (src) + 2 fp8-DR MMs; out_cb(f, psum) evicts (must scale by 1/SW).

        Product order T1a,T1b,P2a,P2b with each support's transposes emitted
        as the lag of the NEXT product (T1a's inside T1b, T1b's inside P2a):
        by the time a transpose issues, its evict input completed a full
        product (~2.7us) earlier, so it never head-blocks the in-order PE
        queue.  PE stalls are extra costly on TRN2 because the tensor
        engine needs ~3us of CONTINUOUS execution to ramp to 2.4GHz (a
        microbench shows back-to-back SWI products sustain 84ns vs the
        ~213ns mid-pstate rate) — keeping the PE stream wait-free keeps it
        ramped."""
        t12s, t1n2s = [], []
        for s in range(2):
            t12s.append(termp.tile([C, 2, N], F8, tag=f"t12_{s}", name=f"t12_{s}"))
            t1n2s.append(tnatp.tile([P, NK // 2, CP], F8, tag=f"tnat{s}",
                                    name=f"t1n2_{s}"))

        def evict_t1(s):
            def ev(f, ps):
                # psum holds SA * A@src; u1 = ST * T1 = psum * (ST/SA).
                # Alternate DVE/ACT like the P2 evicts: products recycle
                # psum banks at the EVICT rate (pp bufs=4 means chunk f's
                # matmuls wait on evict f-4), so the evict stream must run
                # on two engines to keep pace with 0.68us/chunk of matmul.
                fs = slice(f * PFB, (f + 1) * PFB)
                if f % 2 == 0:
                    nc.vector.tensor_scalar_mul(t12s[s][:, 0, fs], ps, ST / SA)
                else:
                    nc.scalar.activation(t12s[s][:, 0, fs], ps, AF.Copy,
                                         scale=ST / SA)
            return ev

        def evict_p2(s):
            # psum = SA * A @ u1 = SA*ST * P2; u2 = 2*ST*P2 = psum * (2/SA)
            def ev(f, ps):
                fs = slice(f * PFB, (f + 1) * PFB)
                if f % 2 == 0:
                    nc.vector.tensor_scalar_mul(t12s[s][:, 1, fs], ps, 2.0 / SA)
                else:
                    nc.scalar.activation(t12s[s][:, 1, fs], ps, AF.Copy,
                                         scale=2.0 / SA)
            return ev

        def lag_tr(s):
            def lg(f):
                transpose_to_nat(t12s[s][:, 0, :], t1n2s[s], 1.0, f, True)
            return lg

        product(src_nat, 0, evict_t1(0))
        product(src_nat, 1, evict_t1(1), lag=lag_tr(0))
        product(t1n2s[0], 0, evict_p2(0), lag=lag_tr(1))
        product(t1n2s[1], 1, evict_p2(1))

        od = wsrc.shape[1]
        for f in range(NF):
            fs = slice(f * FB, (f + 1) * FB)
            ps = dp.tile([G, FB], F32, tag="dp", name="ps_dense")
            nc.tensor.matmul(ps[0:od, :], wsrc, src_T[:, fs],
                             start=True, stop=False)
            for s in range(2):
                nc.tensor.matmul(
                    ps[0:od, :], wpair[:, :, s * od:(s + 1) * od],
                    t12s[s][:, :, fs],
                    start=False, stop=(s == 1), perf_mode=DR,
                )
            out_cb(f, ps[0:od, :])

    # ---- AGCN 1: gates --------------------------------------------------
    zz = const.tile([G, N], BF16)
    agcn(
        xnat, xT, gsrc, gpair,
        lambda f, ps: nc.scalar.activation(
            zz[:, f * FB:(f + 1) * FB], ps, AF.Sigmoid, bias=gateb,
            scale=1.0 / SW,
        ),
    )

    # ---- candidate (chunked per F so transposes pipeline behind gate;
    # transposes trail the DVE chunks by TWO chunks so they reach the PE
    # queue with their inputs already written) -----------------------------
    candT = termp.tile([C, N], BF16, tag="cand", name="candT")
    cnat = natp.tile([P, NK // 2, CP], F8, tag="cnat", name="cnat")

    def cand_chunk(f):
        fs = slice(f * PFB, (f + 1) * PFB)
        nc.vector.tensor_mul(candT[0:DO, fs], zz[0:DO, fs], gin[0:DO, fs])
        nc.vector.tensor_mul(candT[DO:2 * DO, fs], zz[DO:2 * DO, fs], gin[DO:2 * DO, fs])
        nc.vector.tensor_copy(candT[2 * DO:C, fs], gin[2 * DO:C, fs])

    cand_chunk(0)
    cand_chunk(1)
    for f in range(2, NPF):
        cand_chunk(f)
        transpose_to_nat(candT, cnat, 1.0, f - 2, False)
    transpose_to_nat(candT, cnat, 1.0, NPF - 2, False)
    transpose_to_nat(candT, cnat, 1.0, NPF - 1, False)

    # ---- AGCN 2: candidate hc ------------------------------------------
    hc = f32p.tile([DO, N], BF16, tag="mrhc", name="hc")
    agcn(
        cnat, candT, usrc, upair,
        lambda f, ps: nc.scalar.activation(
            hc[:, f * FB:(f + 1) * FB], ps, AF.Tanh, bias=updb,
            scale=1.0 / SW,
        ),
    )

    # ---- h = r*state + (1-r)*hc = hc + r*(state-hc); then hop matmul ----
    # all chunked per F so the tail pipelines with the upd matmuls.
    # r lives at partitions 64:96 of zz; DMA-shift it to 0:32.
    # rT and the output DMAs ride the ACT HWDGE ring: the SP ring is FIFO,
    # so putting them there would block the NEXT iteration's input loads
    # behind this iteration's full compute.
    rT = const.tile([DO, N], BF16)
    nc.scalar.dma_start(out=rT, in_=zz[2 * DO:3 * DO, :])
    h = f32p.tile([DO, N], BF16, tag="h", name="h")
    tn = f32p.tile([DO, N], BF16, tag="state_tn", name="tn")
    for f in range(NPF):
        fs = slice(f * PFB, (f + 1) * PFB)
        nc.vector.tensor_sub(h[:, fs], state[:, fs], hc[:, fs])
        nc.vector.tensor_mul(h[:, fs], h[:, fs], rT[:, fs])
        nc.vector.tensor_add(h[:, fs], h[:, fs], hc[:, fs])
        ps = pp.tile([DO, PFB], F32, tag="pp", name="ps_hop")
        nc.tensor.matmul(ps, hopw, h[:, fs], start=True, stop=True)
        nc.vector.tensor_scalar_add(tn[:, fs], ps, hopb)
    nc.scalar.dma_start(out=hT_d[:, :], in_=h[:, :])
    nc.scalar.dma_start(out=tnT_d[:, :], in_=tn[:, :])


def _get_nc():
    global _NC_CACHE
    if _NC_CACHE is None:
        _NC_CACHE = _build_bass()
    return _NC_CACHE


def _host_prep(inputs):
    bf = ml_dtypes.bfloat16
    f8 = ml_dtypes.float8_e4m3
    xt = np.asarray(inputs["xt"], np.float32)
    s1 = np.asarray(inputs["state1"], np.float32)
    s2 = np.asarray(inputs["state2"], np.float32)
    ge = np.asarray(inputs["gatembedding"], np.float32)
    sup = np.asarray(inputs["supports"], np.float32)

    # feature order on device is [s2, s1, xt] (s2 first so `state` DVE ops
    # are partition-0 aligned); permute weight rows to match.  Gate output
    # columns are permuted to [z2, z1, r] so cand = zz[0:64] * gin[0:64].
    perm66 = np.concatenate([np.arange(34, 66), np.arange(2, 34), np.arange(0, 2)])
    perm74 = np.concatenate([perm66, np.arange(66, 74)])
    zperm = np.concatenate([np.arange(DO, 2 * DO), np.arange(0, DO),
                            np.arange(2 * DO, 3 * DO)])

    def dense_blocks(w, od, colperm, rev_u1, rev_u2):
        """w [6*C, od] -> (src bf16 [C, od], pair f8 [C, 2, 2*od]).
        src slot = SW*(W0a + W0b - W2a - W2b)  (T2 = 2*P2 - X fold);
        pair[:, 0, s] = SW*W1s/ST, pair[:, 1, s] = SW*2*W2s/(2*ST).
        rev_u1/rev_u2: that slot's psum came out feature-REVERSED (SWI
        stationary layout), so reverse its weight rows to match."""
        blk = [w[i * C:(i + 1) * C][perm66][:, colperm] for i in range(6)]
        src = (SW * (blk[0] + blk[3] - blk[2] - blk[5])).astype(bf)
        r1 = slice(None, None, -1) if rev_u1 else slice(None)
        r2 = slice(None, None, -1) if rev_u2 else slice(None)
        pair = np.zeros((C, 2, 2 * od), np.float32)
        pair[:, 0, 0:od] = SW / ST * blk[1][r1]
        pair[:, 1, 0:od] = SW / ST * blk[2][r2]
        pair[:, 0, od:2 * od] = SW / ST * blk[4][r1]
        pair[:, 1, od:2 * od] = SW / ST * blk[5][r2]
        return src, pair.reshape(C, 4 * od).astype(f8)

    # gate: T1 psum normal, P2 psum reversed; upd: T1' reversed, P2' normal
    gsrc, gpair = dense_blocks(np.asarray(inputs["gate_w"], np.float32), G,
                               zperm, rev_u1=False, rev_u2=True)
    usrc, upair = dense_blocks(np.asarray(inputs["upd_w"], np.float32), DO,
                               np.arange(DO), rev_u1=True, rev_u2=False)
    shared = {
        "mlp_w": np.asarray(inputs["mlp_w"], np.float32)[perm74].astype(bf),
        "gsrc": gsrc, "gpair": gpair, "usrc": usrc, "upair": upair,
        "hop_w": np.asarray(inputs["hop_w"], np.float32).astype(bf),
        "mlp_b": np.asarray(inputs["mlp_b"], np.float32).reshape(DO, 1),
        "gate_b": np.asarray(inputs["gate_b"], np.float32)[zperm].reshape(G, 1),
        "upd_b": np.asarray(inputs["upd_b"], np.float32).reshape(DO, 1),
        "hop_b": np.asarray(inputs["hop_b"], np.float32).reshape(DO, 1),
    }
    in_maps = []
    for b in range(NCORES):
        x_cat = np.concatenate([s2[b], s1[b], xt[b]], axis=-1)        # [N, 66]
        gin_cat = np.concatenate([s2[b], s1[b], xt[b], ge[b]], axis=-1)
        # aT / xnat shipped pre-chunked in SBUF-tile layout [P, NK, ...]
        # (flat per partition) so the device loads each in ONE coalesced DMA.
        aT = (sup[:, b].transpose(0, 2, 1) * SA).astype(f8)           # [2, N, N]
        aT = np.ascontiguousarray(
            aT.reshape(2, NK, P, N).transpose(0, 2, 1, 3)
        ).reshape(2, P, NK * N)
        # SWI stationary: feature c of chunk-plane i at byte 2*(127-c)+i
        # (tail-packed, reversed) -> T1 psum partition c in normal order
        xc = x_cat.astype(f8).reshape(NK, P, C).transpose(1, 0, 2)  # [P,NK,C]
        xnat = np.zeros((P, NK // 2, CP), f8)
        for kp in range(NK // 2):
            A, B = xc[:, 2 * kp, ::-1], xc[:, 2 * kp + 1, ::-1]
            xnat[:, kp, 124:CP] = np.stack([A, B], axis=-1).reshape(P, 2 * C)
        xnat = xnat.reshape(P, (NK // 2) * CP)
        in_maps.append({
            "aT": aT,
            "xnat": xnat,
            "gin": np.ascontiguousarray(gin_cat.T).astype(bf),
            "sdiff": np.ascontiguousarray((s1[b] - s2[b]).T).astype(bf),
            **shared,
        })
    return in_maps


def _run(inputs, **kw):
    # Under axon, BASS_TRACE=1 makes run_bass_kernel_spmd import the NTFF
    # hook module, which trimmed containers lack; fail soft to no-trace.
    try:
        from concourse._compat import axon_active
        if axon_active():
            import antenv.axon_hooks  # noqa: F401
    except ImportError:
        import os
        os.environ.setdefault("BASS_NEVER_TRACE", "1")
    nc = _get_nc()
    in_maps = _host_prep(inputs)
    res = run_bass_kernel_spmd(nc, in_maps, core_ids=list(range(NCORES)), **kw)
    h = np.stack([np.asarray(r["h_T"], np.float32).T for r in res.results])
    tn = np.stack([np.asarray(r["tn_T"], np.float32).T for r in res.results])
    return (h, tn), res


def kernel(**inputs):
    return _run(inputs)[0]
